# revision 1
# baseline (speedup 1.0000x reference)
import numpy as np
from contextlib import ExitStack

import concourse.bass as bass
import concourse.tile as tile
from concourse import bacc, mybir
from concourse.bass_utils import run_bass_kernel_spmd

N, C, H, W = 256, 3, 256, 256
D = C * H * W          # 196608
NCORES = 8
RPC = N // NCORES      # 32 rows per core
Q = 4                  # quarters of a row per partition group
P = 128                # partitions = Q * RPC
DPP = D // Q           # 49152 elements per partition
EPS = 1e-6

CHUNKS = [2048, 2048] + [4096] * 10 + [2048, 2048]
assert sum(CHUNKS) == DPP
NCH = len(CHUNKS)

# Per-chunk engine split of the 5 row stats (Sz, Sb, Szz, Sbb, Szb).
# The z*b product (stt) must be on DVE; the rest balances
# DVE (~4.4us / 4096-chunk op) vs ACT (~3.98us).
# r0: DVE {z, zb, b}, ACT {zz, bb}
# r1: DVE {z, zb},    ACT {zz, bb, b}
# r2: DVE {zb, b},    ACT {zz, z, bb}
PATTERN = ["r0", "r1", "r2", "r0", "r1", "r2", "r0", "r1", "r2",
           "r0", "r1", "r2", "r0", "r0"]
assert len(PATTERN) == NCH
N_ZD = sum(r in ("r0", "r1") for r in PATTERN)
N_BD = sum(r in ("r0", "r2") for r in PATTERN)
N_ZA = sum(r == "r2" for r in PATTERN)
N_BA = sum(r == "r1" for r in PATTERN)

_NC = None


def _build_nc():
    fp32 = mybir.dt.float32
    AF = mybir.ActivationFunctionType
    ALU = mybir.AluOpType
    AX = mybir.AxisListType

    nc = bacc.Bacc()
    z_ext = nc.dram_tensor("z", [P, DPP], fp32, kind="ExternalInput")
    b_ext = nc.dram_tensor("b", [P, DPP], fp32, kind="ExternalInput")
    out_ext = nc.dram_tensor("out", [P, 7], fp32, kind="ExternalOutput")

    with tile.TileContext(nc) as tc, ExitStack() as ctx:
        zp = ctx.enter_context(tc.tile_pool(name="zp", bufs=3))
        bp = ctx.enter_context(tc.tile_pool(name="bp", bufs=3))
        dp = ctx.enter_context(tc.tile_pool(name="dp", bufs=2))  # DVE scratch
        ap = ctx.enter_context(tc.tile_pool(name="ap", bufs=2))  # ACT scratch
        acc = ctx.enter_context(tc.tile_pool(name="acc", bufs=1))

        zb_d = acc.tile([P, NCH], fp32)
        zz_a = acc.tile([P, NCH], fp32)
        bb_a = acc.tile([P, NCH], fp32)
        z_d = acc.tile([P, N_ZD], fp32)
        z_a = acc.tile([P, N_ZA], fp32)
        b_d = acc.tile([P, N_BD], fp32)
        b_a = acc.tile([P, N_BA], fp32)
        stats = acc.tile([P, 7], fp32)

        iz_d = iz_a = ib_d = ib_a = 0
        off = 0
        for i, (sz, r) in enumerate(zip(CHUNKS, PATTERN)):
            zt = zp.tile([P, sz], fp32)
            nc.sync.dma_start(zt[:], z_ext[:, off:off + sz])
            bt = bp.tile([P, sz], fp32)
            nc.sync.dma_start(bt[:], b_ext[:, off:off + sz])
            off += sz

            # DVE ops (z-only first so they can start before b lands)
            if r in ("r0", "r1"):
                nc.vector.tensor_reduce(
                    out=z_d[:, iz_d:iz_d + 1], in_=zt[:], axis=AX.X,
                    op=ALU.add)
                iz_d += 1
            prod = dp.tile([P, sz], fp32)
            nc.vector.scalar_tensor_tensor(
                out=prod[:], in0=zt[:], scalar=1.0, in1=bt[:],
                op0=ALU.mult, op1=ALU.mult, accum_out=zb_d[:, i:i + 1])
            if r in ("r0", "r2"):
                nc.vector.tensor_reduce(
                    out=b_d[:, ib_d:ib_d + 1], in_=bt[:], axis=AX.X,
                    op=ALU.add)
                ib_d += 1

            # ACT ops (z-only first)
            scr = ap.tile([P, sz], fp32)
            nc.scalar.activation(out=scr[:], in_=zt[:], func=AF.Square,
                                 accum_out=zz_a[:, i:i + 1])
            if r == "r2":
                nc.scalar.activation(out=scr[:], in_=zt[:], func=AF.Copy,
                                     accum_out=z_a[:, iz_a:iz_a + 1])
                iz_a += 1
            nc.scalar.activation(out=scr[:], in_=bt[:], func=AF.Square,
                                 accum_out=bb_a[:, i:i + 1])
            if r == "r1":
                nc.scalar.activation(out=scr[:], in_=bt[:], func=AF.Copy,
                                     accum_out=b_a[:, ib_a:ib_a + 1])
                ib_a += 1

        # stats cols: [zb, z_d, b_d, zz, bb, z_a, b_a]
        nc.vector.tensor_reduce(out=stats[:, 0:1], in_=zb_d[:], axis=AX.X, op=ALU.add)
        nc.vector.tensor_reduce(out=stats[:, 1:2], in_=z_d[:], axis=AX.X, op=ALU.add)
        nc.vector.tensor_reduce(out=stats[:, 2:3], in_=b_d[:], axis=AX.X, op=ALU.add)
        nc.vector.tensor_reduce(out=stats[:, 3:4], in_=zz_a[:], axis=AX.X, op=ALU.add)
        nc.vector.tensor_reduce(out=stats[:, 4:5], in_=bb_a[:], axis=AX.X, op=ALU.add)
        nc.vector.tensor_reduce(out=stats[:, 5:6], in_=z_a[:], axis=AX.X, op=ALU.add)
        nc.vector.tensor_reduce(out=stats[:, 6:7], in_=b_a[:], axis=AX.X, op=ALU.add)
        nc.sync.dma_start(out_ext[:], stats[:])

    nc.finalize()
    return nc


def _get_nc():
    global _NC
    if _NC is None:
        _NC = _build_nc()
    return _NC


def _shard(x):
    # [RPC, D] row block -> [P, DPP] where partition p = q*RPC + r owns
    # x[r, q*DPP:(q+1)*DPP]
    return np.ascontiguousarray(
        x.reshape(RPC, Q, DPP).transpose(1, 0, 2).reshape(P, DPP))


def kernel(preds, targets, _trace=False):
    preds = np.ascontiguousarray(preds, dtype=np.float32).reshape(N, D)
    targets = np.ascontiguousarray(targets, dtype=np.float32).reshape(N, D)

    in_maps = []
    for c in range(NCORES):
        rows = slice(c * RPC, (c + 1) * RPC)
        in_maps.append({"z": _shard(targets[rows]), "b": _shard(preds[rows])})

    res = run_bass_kernel_spmd(_get_nc(), in_maps, list(range(NCORES)),
                               trace=_trace)
    raw = np.stack([res.results[c]["out"] for c in range(NCORES)])  # [8,P,7]
    raw = raw.astype(np.float64)
    S5 = np.stack([
        raw[..., 1] + raw[..., 5],   # Sz
        raw[..., 2] + raw[..., 6],   # Sb
        raw[..., 3],                 # Szz
        raw[..., 4],                 # Sbb
        raw[..., 0],                 # Szb
    ], axis=-1)
    S = S5.reshape(NCORES, Q, RPC, 5).sum(axis=1).reshape(N, 5)
    Sz, Sb, Szz, Sbb, Szb = (S[:, j] for j in range(5))
    num = Szb - Sz * Sb / D
    vz = Szz - Sz * Sz / D
    vb = Sbb - Sb * Sb / D
    corr = num / (np.sqrt(vz) * np.sqrt(vb) + EPS)
    out = np.array(corr.mean(), dtype=np.float32)
    if _trace:
        return out, res
    return out

